# revision 1
# baseline (speedup 1.0000x reference)
"""Trainium2 Bass kernel for BuildVolume2d (stereo cost volume, L1 over channels).

cost[b, d, h, w] = sum_c |feat_l[b,c,h,w] - feat_r[b,c,h,4w-d]|   (feat_r zero-padded left)

Sharding: batch B=8 -> 8 NeuronCores (data parallel, one sample per core).

Per-core layout (sample b):
  - Iterate over 64 h-groups of 4 rows. SBUF partitions = (h_in_group*32 + c);
    the host pre-transposes inputs to [(h c), w] so each group load is one flat DMA.
  - feat_r row block cast to fp16 and phase-split into Rall tile:
      Rall[:, 524*t + pad_t + j] = r[c,h,4j+t],  pad_0=11, pad_{1,2,3}=12, zeros in pads.
    For disparity d = 4q+s: shifted_r col = R_{(4-s)%4}[w - q - (s>0)], which for all
    4 phases is Rall[11 - q + 524*t + w]  (t in 0..3, d = 4q + perm[t], perm=[0,3,2,1]).
  - 12 mega subtracts per h-group (one per q): diff[128,(4t),(512w)] fp16 (DVE 2x mode).
  - |x| via relu pair: pos = max(x,0) (DVE tensor_scalar 4x), neg part either
    relu(-x) on ACT (reduced with +ones) or min(x,0) on DVE (reduced with -ones);
    the two matmuls accumulate into the same PSUM slot.
  - PSUM drained via ACT copy [96,2048] -> SBUF staging -> DMA to HBM.
"""
import sys
sys.path.insert(0, '/opt/trn_rl_repo')

import numpy as np
import concourse.bass as bass
import concourse.tile as tile
from concourse import bacc, mybir
from concourse.bass_utils import run_bass_kernel_spmd

# ---- problem constants (hardcoded per spec) ----
B, C, H, W = 8, 32, 256, 512
W4 = 4 * W
D = 48                     # maxdisp
N_CORES = 8
HG = 4                     # h rows per group
N_HG = H // HG             # 64
PW = 524                   # per-phase block width in Rall
RALL_W = 4 * PW            # 2096
RALL_ALLOC = RALL_W + 12   # slack so the q-shifted window slice stays in range
PERM = [0, 3, 2, 1]        # t -> s so that d = 4q + PERM[t]

f32 = mybir.dt.float32
fp16 = mybir.dt.float16

# engine assignment tunables (counts per h-group, out of 12 q-instructions).
# GpSimd shares an SBUF port pair with the DVE: giving it tensor work knocks
# DVE tensor_scalar from 4x to 2x mode, so it only does tiny memsets.
N_ACT_ABS = 6              # q's reduced via ACT activation(Abs) + one matmul set;
                           # the rest use the DVE relu/min pair + two matmul sets

_compiled = None


def build_program(n_hg=N_HG):
    nc = bacc.Bacc("TRN2", target_bir_lowering=False, debug=False, num_devices=N_CORES)
    # host pre-transposes to h-major rows: [(h c), w]
    fl = nc.dram_tensor("feat_l", [H * C, W], f32, kind="ExternalInput").ap()
    fr = nc.dram_tensor("feat_r", [H * C, W4], f32, kind="ExternalInput").ap()
    ones = nc.dram_tensor("ones_st", [128, 32], fp16, kind="ExternalInput").ap()
    onesn = nc.dram_tensor("ones_neg", [128, 32], fp16, kind="ExternalInput").ap()
    out = nc.dram_tensor("cost", [D, H, W], f32, kind="ExternalOutput").ap()

    with tile.TileContext(nc) as tc:
        with (
            tc.tile_pool(name="const", bufs=1) as constp,
            tc.tile_pool(name="inp", bufs=4) as inp,
            tc.tile_pool(name="r16p", bufs=3) as r16p,
            tc.tile_pool(name="l16p", bufs=3) as l16p,
            tc.tile_pool(name="diffp", bufs=6) as diffp,
            tc.tile_pool(name="absp", bufs=6) as absp,
            tc.tile_pool(name="stgp", bufs=4) as stgp,
            tc.tile_pool(name="psum", bufs=2, space="PSUM") as psp,
        ):
            ost = constp.tile([128, 32], fp16, name="ost")
            nc.sync.dma_start(ost[:], ones[:])
            ostn = constp.tile([128, 32], fp16, name="ostn")
            nc.sync.dma_start(ostn[:], onesn[:])

            for g in range(n_hg):
                h0 = HG * g
                # ---- load ----
                lf32 = inp.tile([128, W], f32, name="lf32", tag="lf32")
                nc.sync.dma_start(lf32[:], fl[128 * g:128 * (g + 1), :])
                rf32 = inp.tile([128, W4], f32, name="rf32", tag="rf32")
                nc.sync.dma_start(rf32[:], fr[128 * g:128 * (g + 1), :])

                # ---- casts ----
                l16 = l16p.tile([128, W], fp16, name="l16")
                nc.vector.tensor_copy(l16[:], lf32[:])

                rall = r16p.tile([128, RALL_ALLOC], fp16, name="rall")
                # zero pads: [0:11], [523:536], [1047:1060], [1571:1584]
                nc.gpsimd.memset(rall[:, 0:11], 0.0)
                nc.gpsimd.memset(rall[:, 523:536], 0.0)
                nc.gpsimd.memset(rall[:, 1047:1060], 0.0)
                nc.gpsimd.memset(rall[:, 1571:1584], 0.0)
                for t in range(4):
                    base = PW * t + (11 if t == 0 else 12)
                    src = rf32[:, t:W4:4]
                    dst = rall[:, base:base + W]
                    nc.scalar.copy(dst, src)

                # ---- per-fill loop: 4 fills x 3 q x 4 t ----
                for F in range(4):
                    pt = psp.tile([128, 2048], f32, name="pt")
                    for qi in range(3):
                        q = 3 * F + qi
                        dif = diffp.tile([128, 4, W], fp16, name="dif")
                        in0 = l16[:].unsqueeze(1).broadcast_to((128, 4, W))
                        in1 = rall[:, 11 - q: 11 - q + RALL_W] \
                            .rearrange("p (t w) -> p t w", t=4)[:, :, :W]
                        nc.vector.tensor_tensor(
                            dif[:], in0, in1, op=mybir.AluOpType.subtract)

                        d2 = dif[:].rearrange("p t w -> p (t w)")
                        if q in _ACT_ABS_QS:
                            ab = absp.tile([128, 4, W], fp16, name="ab", tag="ab")
                            nc.scalar.activation(
                                ab[:].rearrange("p t w -> p (t w)"), d2,
                                mybir.ActivationFunctionType.Abs)
                            for t in range(4):
                                fslot = PERM[t]
                                nc.tensor.matmul(
                                    pt[32 * qi:32 * qi + 32,
                                       512 * fslot:512 * fslot + 512],
                                    ost[:], ab[:, t, :], start=True, stop=True)
                        else:
                            pos = absp.tile([128, 4, W], fp16, name="pos", tag="pos")
                            nc.vector.tensor_scalar_max(
                                pos[:].rearrange("p t w -> p (t w)"), d2, 0.0)
                            neg = absp.tile([128, 4, W], fp16, name="neg", tag="neg")
                            nc.vector.tensor_scalar_min(
                                neg[:].rearrange("p t w -> p (t w)"), d2, 0.0)
                            for t in range(4):
                                fslot = PERM[t]
                                dst = pt[32 * qi:32 * qi + 32,
                                         512 * fslot:512 * fslot + 512]
                                nc.tensor.matmul(dst, ost[:], pos[:, t, :],
                                                 start=True, stop=False)
                                nc.tensor.matmul(dst, ostn[:], neg[:, t, :],
                                                 start=False, stop=True)

                    # drain psum -> staging
                    stg = stgp.tile([128, 2048], f32, name="stg")
                    nc.scalar.copy(stg[0:96, :], pt[0:96, :])
                    # out DMA: base b holds d = 12F + 4b + f, partitions 32b+h
                    for b in range(3):
                        d0 = 12 * F + 4 * b
                        nc.sync.dma_start(
                            out[d0:d0 + 4, h0:h0 + HG, :].rearrange("d h w -> h d w"),
                            stg[32 * b:32 * b + 4, :].rearrange("h (d w) -> h d w", d=4))
    nc.compile()
    return nc


_ACT_ABS_QS = set(range(0, 12, 2))[:N_ACT_ABS] if False else set(
    q for q in range(12) if q % 2 == 0)


def make_ones():
    # partition k = h*32 + c; output row m carries h == m % 4 (8 replicas so
    # every PSUM row in the 32-row group is written; DMA reads rows 0..3).
    on = np.zeros((128, 32), np.float16)
    for m in range(32):
        h = m % 4
        on[h * 32:(h + 1) * 32, m] = 1.0
    return on


def prep_in_maps(feat_l, feat_r):
    on = make_ones()
    onn = -on
    maps = []
    for i in range(N_CORES):
        flt = np.ascontiguousarray(
            feat_l[i].transpose(1, 0, 2)).reshape(H * C, W)
        frt = np.ascontiguousarray(
            feat_r[i].transpose(1, 0, 2)).reshape(H * C, W4)
        maps.append({"feat_l": flt, "feat_r": frt, "ones_st": on,
                     "ones_neg": onn})
    return maps


def kernel(feat_l, feat_r, maxdisp):
    global _compiled
    feat_l = np.asarray(feat_l, dtype=np.float32)
    feat_r = np.asarray(feat_r, dtype=np.float32)
    assert int(maxdisp) == D
    assert feat_l.shape == (B, C, H, W) and feat_r.shape == (B, C, H, W4)
    if _compiled is None:
        _compiled = build_program()
    in_maps = prep_in_maps(feat_l, feat_r)
    res = run_bass_kernel_spmd(_compiled, in_maps, list(range(N_CORES)))
    return np.stack([res.results[i]["cost"] for i in range(N_CORES)], axis=0)
